# revision 4
# baseline (speedup 1.0000x reference)
"""EPLL MoE-routing kernel for 8 trn2 NeuronCores — fp8 DoubleRow version.

Per launch (one per beta), per core (8192 patches):
- Host: builds augmented quadratic-form operands; rows importance-sorted;
  M (coefficients) mean-centered over k, pow2 row-scaled, quantized fp8e4m3
  with a hi+lo two-term expansion for the top NTOP rows; patch features
  (outer products) quantized fp8e4m3.
- Device: per 128-patch tile, fp8 DoubleRow matmuls accumulate
  logpost[patch, k] into PSUM fp32; copy-cast to int16 (runtime scale) on
  scalar/vector/gpsimd round-robin; big grouped DMAs out.
- Host: argmax over k, Wiener apply, overlap-add, blend.

Self-contained: shapes hardcoded for y[1,1,256,256], K=200, D=36.
"""

import sys

sys.path.insert(0, "/opt/trn_rl_repo")

import numpy as np
import ml_dtypes

B, C, H, W = 1, 1, 256, 256
PS = 6
K = 200
D = PS * PS * C            # 36
SIGMA_SQ = 0.01
BETAS = [b / SIGMA_SQ for b in (1.0, 4.0, 8.0, 16.0, 32.0)]
NPIX = C * H * W

NI = H - PS + 1            # 251
P = NI * NI                # 63001
N_CORES = 8
PPAD = 63488               # 8 * 7936 >= P
PPC = PPAD // N_CORES      # 7936 patches per core
NTILE = PPC // 128         # 62 tiles of 128 patches
NSYM = D * (D + 1) // 2    # 666
NROW = NSYM + D + 1        # 703 augmented rows
NROWP = 704                # padded: 256 + 256 + 192
NCOR = 64                  # SVD error-correction rows (chunk2 spare slots)

_IU, _IV = np.triu_indices(D)
_SYM_SCALE = np.where(_IU == _IV, 1.0, 2.0).astype(np.float32)

F8 = ml_dtypes.float8_e4m3


def _patch_linear_indices():
    i0 = np.arange(NI)
    rows = i0[:, None, None, None] + np.arange(PS)[None, None, :, None]
    cols = i0[None, :, None, None] + np.arange(PS)[None, None, None, :]
    return (rows * W + cols).reshape(NI * NI, PS * PS).astype(np.int64)


LIN = _patch_linear_indices()          # [P, D]

_STATE = {}


# NOTE: gpsimd (Pool) cannot read PSUM on real HW (BIR verifier), so
# copies only go to Act (0) and DVE (1); Pool is a DMA queue.
DEFAULT_CFG = {
    "SLABS": [512, 512, 768, 1024, 2048, 1536, 512, 768, 256],
    "SLABQ": [2, 0, 1, 0, 2, 0, 2, 0, 0],   # 0=sync 1=scalar 2=gpsimd
    "NWARM": 14,                             # PE p-state warmup matmuls
    "GRP": 2,                                # PSUM tiles per copy group
    "OGROUPS": [20, 16, 12, 6, 4, 4],        # tiles per output DMA
    "OUTQ": [2, 2, 0, 2, 1, 0],
    "CPQ": [0, 1, 0, 1, 1, 0, 1, 1, 0, 1, 0, 0, 0, 1, 0, 0,
            1, 0, 1, 0, 1, 1, 1, 0, 1, 1, 0, 1, 0, 0, 0],
    "BUFS": 4,
}


def _build_bass(cfg=None):
    from concourse import bacc, mybir
    from concourse.tile import TileContext

    cfg = dict(DEFAULT_CFG, **(cfg or {}))

    nc = bacc.Bacc("TRN2", target_bir_lowering=False, debug=False,
                   num_devices=N_CORES)

    f8 = mybir.dt.float8e4
    DR = mybir.MatmulPerfMode.DoubleRow

    # features (stationary): [p, 2c+i, patch], logical row = 256c+128i+p
    # (chunk 2 occupies partitions 0:96 of slots 4:6; rest is padding)
    ot_dram = nc.dram_tensor("ot", [128, 6, PPC], f8, kind="ExternalInput")
    # mpar: slots 0:6 = M_hi chunks (chunk2 rows 192:256 hold the SVD
    # error-correction coefficients) — one early DMA
    mpar_dram = nc.dram_tensor("mpar", [128, 6, K], f8, kind="ExternalInput")
    lp_dram = nc.dram_tensor("lp", [128, NTILE, K], mybir.dt.int16,
                             kind="ExternalOutput")
    sc_dram = nc.dram_tensor("sc", [128, 1], mybir.dt.float32,
                             kind="ExternalInput")

    SLABS = cfg["SLABS"]
    SLABQ = cfg["SLABQ"]
    NWARM = cfg["NWARM"]
    GRP = cfg["GRP"]
    OGROUPS = cfg["OGROUPS"]
    OUTQ = cfg["OUTQ"]
    CPQ = cfg["CPQ"]

    with TileContext(nc) as tc:
        with (
            tc.tile_pool(name="apool", bufs=1) as apool,
            tc.tile_pool(name="psum", bufs=cfg["BUFS"], space="PSUM") as pspool,
        ):
            mpar_sb = apool.tile([128, 6, K], f8)
            nc.sync.dma_start(mpar_sb[:], mpar_dram.ap())
            sc_sb = apool.tile([128, 1], mybir.dt.float32)
            nc.sync.dma_start(sc_sb[:], sc_dram.ap())

            ot_sb = apool.tile([128, 6, PPC], f8)
            lp_sb = apool.tile([128, NTILE, K], mybir.dt.int16)

            mh = mpar_sb

            in_q = [nc.sync, nc.scalar, nc.gpsimd]
            off = 0
            for s, width in enumerate(SLABS):
                sl = slice(off, off + width)
                in_q[SLABQ[s]].dma_start(ot_sb[:, :, sl],
                                         ot_dram.ap()[:, :, sl])
                off += width

            # PE p-state warmup: dependency-free junk DoubleRow matmuls on
            # a zeroed tile (dual-fp8 Ldweights needs a clean stationary
            # AP); starts right after the memset, covering the ramp window
            wsb = apool.tile([128, 2, 128], f8)
            nc.vector.memset(wsb[:], 0)
            wps = pspool.tile([128, GRP, 512], mybir.dt.float32, tag="ps")
            for w in range(NWARM):
                nc.tensor.matmul(wps[:, 0, 0:128], wsb[:], wsb[:],
                                 start=True, stop=True, perf_mode=DR)

            cp_engines = [nc.scalar, nc.vector, nc.gpsimd]
            out_qs = [nc.sync, nc.scalar, nc.gpsimd]
            t = 0
            gi = 0
            for og, gw in enumerate(OGROUPS):
                g0 = t
                for _ in range(gw // GRP):
                    ps = pspool.tile([128, GRP, 512], mybir.dt.float32,
                                     tag="ps")
                    for j in range(GRP):
                        tl = slice(t * 128, (t + 1) * 128)
                        out = ps[:, j, 0:K]
                        nc.tensor.matmul(out, ot_sb[:, 0:2, tl],
                                         mh[:, 0:2, :],
                                         start=True, stop=False, perf_mode=DR)
                        nc.tensor.matmul(out, ot_sb[:, 2:4, tl],
                                         mh[:, 2:4, :],
                                         start=False, stop=False, perf_mode=DR)
                        nc.tensor.matmul(out, ot_sb[:, 4:6, tl],
                                         mh[:, 4:6, :],
                                         start=False, stop=True, perf_mode=DR)
                        t += 1
                    eng = cp_engines[CPQ[gi]]
                    gi += 1
                    src = ps[:, :, 0:K]
                    dst = lp_sb[:, t - GRP:t, :]
                    if eng is nc.scalar:
                        eng.mul(dst, src, sc_sb[:, 0:1])
                    else:
                        eng.tensor_scalar_mul(dst, src, sc_sb[:, 0:1])
                out_qs[OUTQ[og]].dma_start(
                    lp_dram.ap()[:, g0:t, :], lp_sb[:, g0:t, :])
    nc.finalize()
    return nc


def _get_state():
    if not _STATE:
        _STATE["nc"] = _build_bass()
    return _STATE


def _pack_rows(Mrows, rows96=False):
    """[512 or 192, N] -> [128|96, nslots, N] with row r = 128*slot + p."""
    nrow, n = Mrows.shape
    if rows96:
        return np.ascontiguousarray(
            Mrows.reshape(2, 96, n).transpose(1, 0, 2))
    nslot = nrow // 128
    return np.ascontiguousarray(
        Mrows.reshape(nslot, 128, n).transpose(1, 0, 2))


def kernel(y, mu, log_weights, eigvecs, eigvals):
    from concourse import bass_utils

    y = np.asarray(y, np.float32)
    mu = np.asarray(mu, np.float32)
    lw = np.asarray(log_weights, np.float32)
    U = np.asarray(eigvecs, np.float32)
    ev = np.asarray(eigvals, np.float32)

    st = _get_state()
    nc = st["nc"]

    yf = y.reshape(-1)
    x = yf.copy()

    mult = np.bincount(LIN.ravel(), minlength=NPIX).astype(np.float32)
    inv_mult = 1.0 / mult

    for beta in BETAS:
        reg = 1.0 / beta
        l = ev + reg                                        # [K, D]
        il = (1.0 / l).astype(np.float32)
        A = np.einsum("kde,ke,kfe->kdf", U, il, U)          # [K, D, D]
        E = np.einsum("kde,ke,kfe->kdf", U, ev * il, U)     # [K, D, D]
        logdet = np.log(l).sum(1)
        Amu = np.einsum("kdf,kf->kd", A, mu)                # [K, D]
        muAmu = np.einsum("kd,kd->k", mu, Amu)
        cterm = (lw - 0.5 * logdet - 0.5 * muAmu).astype(np.float32)

        M = np.empty((NROWP, K), np.float32)
        M[:NSYM] = (-0.5 * _SYM_SCALE[:, None]
                    * A[:, _IU, _IV].T.astype(np.float32))
        M[NSYM:NSYM + D] = Amu.T
        M[NSYM + D] = cterm
        M[NROW:] = 0.0

        pat = x[LIN]                                        # [P, D]
        OT = np.zeros((NROWP, PPAD), np.float32)
        OT[:NSYM, :P] = (pat[:, _IU] * pat[:, _IV]).T
        OT[NSYM:NSYM + D, :P] = pat.T
        OT[NSYM + D, :P] = 1.0

        # mean-center coefficients over k (argmax-invariant per patch)
        Mc = M - M.mean(axis=1, keepdims=True)

        # importance sort
        imp = Mc.std(axis=1) * np.sqrt((OT ** 2).mean(axis=1))
        order = np.argsort(-imp)
        Mo = Mc[order]
        OTo = OT[order]

        # feature-optimal pow2 scales s_r (row max -> ~112); common product
        # scale G (argmax-invariant); coefficient side gets u_r = G/s_r
        # (e4m3 max 240, so targets 112/224)
        G = 1024.0
        ox = np.abs(OTo).max(axis=1)
        ox[ox == 0] = 1.0
        s = np.exp2(np.floor(np.log2(112.0 / ox))).astype(np.float32)
        u = (G / s).astype(np.float32)
        mx = np.abs(Mo).max(axis=1)
        bad = mx * u > 224.0
        while bad.any():
            s[bad] *= 2.0
            u = (G / s).astype(np.float32)
            bad = mx * u > 224.0

        Mhi8 = (Mo * u[:, None]).astype(F8)
        OT8 = (OTo * s[:, None]).astype(F8)
        assert np.isfinite(Mhi8.astype(np.float32)).all()
        assert np.isfinite(OT8.astype(np.float32)).all()

        # SVD error correction: top-64 components of the coefficient
        # quantization error, as 64 extra contraction rows in chunk2
        dM = (Mhi8.astype(np.float32) / u[:, None]) - Mo     # [704, K]
        Uc, S, Vt = np.linalg.svd(dM[:NROW], full_matrices=False)
        Ucs = (Uc[:, :NCOR] * S[:NCOR]).astype(np.float32)   # [703, 64]
        Vc = Vt[:NCOR].astype(np.float32)                    # [64, K]
        fc = Ucs.T @ OTo[:NROW]                              # [64, PPAD]
        fx = np.abs(fc).max(axis=1)
        fx[fx == 0] = 1.0
        scr = np.exp2(np.floor(np.log2(112.0 / fx))).astype(np.float32)
        ucr = (G / scr).astype(np.float32)
        vx = np.abs(Vc).max(axis=1)
        bad2 = vx * ucr > 224.0
        while bad2.any():
            scr[bad2] *= 2.0
            ucr = (G / scr).astype(np.float32)
            bad2 = vx * ucr > 224.0
        Cq8 = (-Vc * ucr[:, None]).astype(F8)                # [64, K]
        Fq8 = (fc * scr[:, None]).astype(F8)                 # [64, PPAD]
        assert np.isfinite(Cq8.astype(np.float32)).all()
        assert np.isfinite(Fq8.astype(np.float32)).all()

        # chunk2: 256 rows = 192 real (511:704, incl zero pad) + 64 corr
        M2 = np.concatenate([Mhi8[512:704], Cq8], axis=0)    # [256, K]
        mh_full = np.empty((128, 6, K), F8)
        mh_full[:, 0:4, :] = _pack_rows(Mhi8[0:512])
        mh_full[:, 4:6, :] = _pack_rows(M2)

        # int16 copy scale from a rigorous Hoelder bound on |PSUM|
        mq = np.abs(Mhi8.astype(np.float32)).max(axis=1)
        oq = np.abs(OT8.astype(np.float32)).max(axis=1)
        bound = float((mq * oq).sum()
                      + (np.abs(Cq8.astype(np.float32)).max(axis=1)
                         * np.abs(Fq8.astype(np.float32)).max(axis=1)).sum())
        sc_in = np.full((128, 1), 32000.0 / (bound * 1.02), np.float32)

        in_maps = []
        for c in range(N_CORES):
            csl = slice(c * PPC, (c + 1) * PPC)
            OT2 = np.concatenate([OT8[512:704, csl], Fq8[:, csl]], axis=0)
            otc = np.empty((128, 6, PPC), F8)
            otc[:, 0:4, :] = _pack_rows(np.ascontiguousarray(OT8[0:512, csl]))
            otc[:, 4:6, :] = _pack_rows(np.ascontiguousarray(OT2))
            in_maps.append({"ot": otc, "mpar": mh_full, "sc": sc_in})

        res = bass_utils.run_bass_kernel_spmd(
            nc, in_maps, core_ids=list(range(N_CORES)))

        ks_parts = []
        for r in res.results:
            lp = r["lp"]                                   # [128, 64, K] i16
            ks_parts.append(lp.argmax(axis=2).T.reshape(-1))  # [8192]
        ks = np.concatenate(ks_parts)[:P]

        est = np.einsum("pde,pe->pd", E[ks], pat)
        xt = np.bincount(LIN.ravel(), weights=est.ravel().astype(np.float64),
                         minlength=NPIX).astype(np.float32)
        xt *= inv_mult
        cdf = beta * SIGMA_SQ
        x = (yf + cdf * xt) / (1.0 + cdf)

    return x.reshape(B, C, H, W).astype(np.float32)
